# revision 9
# baseline (speedup 1.0000x reference)
"""Trainium2 Bass kernel for BaseEnergyFormPhysics (TET4 linear elasticity energy).

Strategy (sharding_hint): shard elements across the 8 NeuronCores; replicate
the nodal table (coords+us interleaved, padded to 32B rows) on every core.
Each core gathers its elements' nodal rows via indirect DMA (128 rows per
instruction, one row per SBUF partition), computes the per-element energy
contribution psi*detJ/6 with DVE ops in fp32, and accumulates per-partition
partial sums. The host sums the 8x128 partials; state_new is a passthrough
of state_old.

Energy formulation (algebraically equal to the reference):
  e_k = x_k - x_0, du_k = u_k - u_0 (k=1..3)
  r_1 = e2 x e3, r_2 = e3 x e1, r_3 = e1 x e2   (adjugate rows * det)
  det = e1 . r1
  P[i][j] = sum_k du_k[i] * r_k[j]              (= grad_u * det)
  psi*det^2 = 0.5*(P00^2+P11^2+P22^2 + 0.5*(t01^2+t02^2+t12^2) + tr(P)^2)
      with t_ij = P_ij + P_ji   [MU=0.5, LAM=1.0]
  contribution = psi*det^2 / (6*det) = num * recip(12*det)
      where num = diag_sq + 0.5*off_sq + tr^2

Degenerate elements (duplicate nodes) give 0 * inf = NaN, matching the
reference (which hits a singular J -> inf/NaN through linalg.inv).
"""

import numpy as np

N_NODES = 800_000
N_ELS = 4_000_000
NC = 8

ELS_PER_CORE = N_ELS // NC            # 500000
QB = 128                              # quads (groups of 128 elements) per batch
BATCHES = 31                          # 31*128*128 = 507904 >= 500000
QUADS = BATCHES * QB                  # 3968
EPC_PAD = QUADS * 128                 # 507904 elements per core, padded
GPB = QB * 4                          # gather instructions per batch (512)
TBL_ROWS = N_NODES + 4                # + unit tet with zero displacement (pads)

_cache = {}


def _configure(n_nodes, n_els, batches):
    """Test hook: shrink the problem (call before kernel())."""
    global N_NODES, N_ELS, ELS_PER_CORE, BATCHES, QUADS, EPC_PAD, TBL_ROWS
    N_NODES = n_nodes
    N_ELS = n_els
    ELS_PER_CORE = N_ELS // NC
    BATCHES = batches
    QUADS = BATCHES * QB
    EPC_PAD = QUADS * 128
    TBL_ROWS = N_NODES + 4
    assert EPC_PAD >= ELS_PER_CORE
    _cache.clear()


def _build_nc():
    import concourse.bass as bass
    import concourse.bacc as bacc
    import concourse.mybir as mybir
    import concourse.tile as tile

    f32 = mybir.dt.float32
    i32 = mybir.dt.int32
    P = 128

    nc = bacc.Bacc("TRN2", target_bir_lowering=False, debug=False)
    table = nc.dram_tensor("table", [TBL_ROWS, 8], f32, kind="ExternalInput").ap()
    idx = nc.dram_tensor("idx", [BATCHES, P, GPB], i32, kind="ExternalInput").ap()
    out = nc.dram_tensor("out", [P, 1], f32, kind="ExternalOutput").ap()

    with tile.TileContext(nc) as tc:
        with (
            tc.tile_pool(name="idxp", bufs=2) as idxp,
            tc.tile_pool(name="gp", bufs=2) as gp,
            tc.tile_pool(name="tp", bufs=1) as tp,
            tc.tile_pool(name="accp", bufs=1) as accp,
        ):
            acc = accp.tile([P, QB], f32)
            nc.vector.memset(acc[:], 0.0)

            def body(iv):
                idxt = idxp.tile([P, GPB], i32)
                nc.sync.dma_start(idxt[:], idx[iv])
                G = gp.tile([P, GPB * 8], f32)
                for j in range(GPB):
                    nc.gpsimd.indirect_dma_start(
                        out=G[:, j * 8 : (j + 1) * 8],
                        out_offset=None,
                        in_=table[:],
                        in_offset=bass.IndirectOffsetOnAxis(
                            ap=idxt[:, j : j + 1], axis=0
                        ),
                        bounds_check=TBL_ROWS - 1,
                        oob_is_err=False,
                    )

                # strided views: value of (slot k, field f) for the batch's
                # 128 quads: [P, QB] with free stride 32
                G3 = G[:].rearrange("p (q w) -> p q w", w=32)

                def V(k, f):
                    return G3[:, :, k * 8 + f]

                # temp pool: unique tag per temp so live ranges don't collide;
                # same tags across batches -> slots reused batch to batch
                tctr = [0]

                def T():
                    tctr[0] += 1
                    return tp.tile(
                        [P, QB], f32, name=f"tmp{tctr[0]}", tag=f"tmp{tctr[0]}"
                    )

                def sub(a, b):
                    o = T()
                    nc.vector.tensor_tensor(o[:], a, b, op=mybir.AluOpType.subtract)
                    return o

                def mul(a, b):
                    o = T()
                    nc.vector.tensor_tensor(o[:], a, b, op=mybir.AluOpType.mult)
                    return o

                def add(a, b):
                    o = T()
                    nc.vector.tensor_tensor(o[:], a, b, op=mybir.AluOpType.add)
                    return o

                # edge vectors e_k = x_k - x_0 ; du_k = u_k - u_0
                e = [[sub(V(k, f), V(0, f)) for f in range(3)] for k in (1, 2, 3)]
                du = [[sub(V(k, 3 + f), V(0, 3 + f)) for f in range(3)] for k in (1, 2, 3)]

                def cross(a, b):
                    o = []
                    for i in range(3):
                        j, k = (i + 1) % 3, (i + 2) % 3
                        m1 = mul(a[j][:], b[k][:])
                        m2 = mul(a[k][:], b[j][:])
                        o.append(sub(m1[:], m2[:]))
                    return o

                r1 = cross(e[1], e[2])   # e2 x e3
                r2 = cross(e[2], e[0])   # e3 x e1
                r3 = cross(e[0], e[1])   # e1 x e2
                r = [r1, r2, r3]

                # det = e1 . r1
                d0 = mul(e[0][0][:], r1[0][:])
                d1 = mul(e[0][1][:], r1[1][:])
                d2 = mul(e[0][2][:], r1[2][:])
                det = add(add(d0[:], d1[:])[:], d2[:])

                # P[i][j] = sum_k du_k[i] r_k[j]
                Pm = []
                for i in range(3):
                    row = []
                    for j in range(3):
                        m1 = mul(du[0][i][:], r[0][j][:])
                        m2 = mul(du[1][i][:], r[1][j][:])
                        m3 = mul(du[2][i][:], r[2][j][:])
                        row.append(add(add(m1[:], m2[:])[:], m3[:]))
                    Pm.append(row)

                # num = diag_sq + 0.5*off_sq + tr^2
                dsq0 = mul(Pm[0][0][:], Pm[0][0][:])
                dsq1 = mul(Pm[1][1][:], Pm[1][1][:])
                dsq2 = mul(Pm[2][2][:], Pm[2][2][:])
                diag = add(add(dsq0[:], dsq1[:])[:], dsq2[:])
                t01 = add(Pm[0][1][:], Pm[1][0][:])
                t02 = add(Pm[0][2][:], Pm[2][0][:])
                t12 = add(Pm[1][2][:], Pm[2][1][:])
                o0 = mul(t01[:], t01[:])
                o1 = mul(t02[:], t02[:])
                o2 = mul(t12[:], t12[:])
                off = add(add(o0[:], o1[:])[:], o2[:])
                # s1 = diag + 0.5*off
                s1 = T()
                nc.vector.scalar_tensor_tensor(
                    s1[:], off[:], 0.5, diag[:],
                    op0=mybir.AluOpType.mult, op1=mybir.AluOpType.add,
                )
                tr = add(add(Pm[0][0][:], Pm[1][1][:])[:], Pm[2][2][:])
                tr2 = mul(tr[:], tr[:])
                num = add(s1[:], tr2[:])

                # contribution = 0.5 * num / (6*det) = num * recip(12*det)
                det12 = T()
                nc.vector.tensor_scalar_mul(det12[:], det[:], 12.0)
                rec = T()
                nc.vector.reciprocal(rec[:], det12[:])
                contrib = mul(num[:], rec[:])
                nc.vector.tensor_tensor(
                    acc[:], acc[:], contrib[:], op=mybir.AluOpType.add
                )

            for b in range(BATCHES):
                body(b)

            red = accp.tile([P, 1], f32)
            nc.vector.tensor_reduce(
                red[:], acc[:], axis=mybir.AxisListType.X, op=mybir.AluOpType.add
            )
            nc.sync.dma_start(out[:], red[:])
    nc.compile()
    return nc


def _prepare_inputs(coords, us, conns):
    """Build the padded node table and per-core slot-major index arrays."""
    table = np.zeros((TBL_ROWS, 8), dtype=np.float32)
    table[:N_NODES, 0:3] = coords
    table[:N_NODES, 3:6] = us
    # pad element nodes: unit tet, zero displacement -> exactly zero energy
    table[N_NODES + 0, 0:3] = (0.0, 0.0, 0.0)
    table[N_NODES + 1, 0:3] = (1.0, 0.0, 0.0)
    table[N_NODES + 2, 0:3] = (0.0, 1.0, 0.0)
    table[N_NODES + 3, 0:3] = (0.0, 0.0, 1.0)

    pad_row = np.array(
        [N_NODES, N_NODES + 1, N_NODES + 2, N_NODES + 3], dtype=np.int32
    )
    idx_maps = []
    for c in range(NC):
        sh = conns[c * ELS_PER_CORE : (c + 1) * ELS_PER_CORE]
        full = np.empty((EPC_PAD, 4), dtype=np.int32)
        full[:ELS_PER_CORE] = sh
        full[ELS_PER_CORE:] = pad_row
        # element (b, q, p) = full[b*16384 + q*128 + p]
        # gather instruction j = q*4 + k gathers partition p <- slot k of elem q,p
        e4 = full.reshape(BATCHES, QB, 128, 4)        # [b, q, p, k]
        idx_arr = e4.transpose(0, 2, 1, 3).reshape(BATCHES, 128, GPB)
        # ^ [b, p, q*4+k] : instruction j=q*4+k, partition p
        idx_maps.append(np.ascontiguousarray(idx_arr))
    return table, idx_maps


def kernel(coords, us, conns, t, state_old, dt):
    from concourse.bass_utils import run_bass_kernel_spmd

    coords = np.asarray(coords)
    us = np.asarray(us)
    conns = np.asarray(conns)

    if "nc" not in _cache:
        _cache["nc"] = _build_nc()
    nc = _cache["nc"]

    table, idx_maps = _prepare_inputs(coords, us, conns)
    in_maps = [{"table": table, "idx": idx_maps[c]} for c in range(NC)]
    res = run_bass_kernel_spmd(nc, in_maps, core_ids=list(range(NC)))
    partials = np.concatenate([r["out"].ravel() for r in res.results])
    pi = np.float32(partials.astype(np.float32).sum(dtype=np.float64))
    if np.isnan(partials).any():
        pi = np.float32(np.nan)
    return pi, np.asarray(state_old)


# revision 13
# speedup vs baseline: 4.7399x; 4.7399x over previous
"""Trainium2 Bass kernel for BaseEnergyFormPhysics (TET4 linear elasticity energy).

Strategy (sharding_hint): shard elements across the 8 NeuronCores; replicate
the nodal table (coords+us interleaved, padded to 32B rows) on every core.
Each core gathers its elements' nodal rows via indirect DMA (128 rows per
instruction, one row per SBUF partition), computes the per-element energy
contribution psi*detJ/6 with DVE ops in fp32, and accumulates per-partition
partial sums. The host sums the 8x128 partials; state_new is a passthrough
of state_old.

Energy formulation (algebraically equal to the reference):
  e_k = x_k - x_0, du_k = u_k - u_0 (k=1..3)
  r_1 = e2 x e3, r_2 = e3 x e1, r_3 = e1 x e2   (adjugate rows * det)
  det = e1 . r1
  P[i][j] = sum_k du_k[i] * r_k[j]              (= grad_u * det)
  psi*det^2 = 0.5*(P00^2+P11^2+P22^2 + 0.5*(t01^2+t02^2+t12^2) + tr(P)^2)
      with t_ij = P_ij + P_ji   [MU=0.5, LAM=1.0]
  contribution = psi*det^2 / (6*det) = num * recip(12*det)
      where num = diag_sq + 0.5*off_sq + tr^2

Degenerate elements (duplicate nodes) give 0 * inf = NaN, matching the
reference (which hits a singular J -> inf/NaN through linalg.inv).
"""

import numpy as np

N_NODES = 800_000
N_ELS = 4_000_000
NC = 8

ELS_PER_CORE = N_ELS // NC            # 500000
QB = 128                              # quads (groups of 128 elements) per batch
BATCHES = 31                          # 31*128*128 = 507904 >= 500000
QUADS = BATCHES * QB                  # 3968
EPC_PAD = QUADS * 128                 # 507904 elements per core, padded
GPB = QB * 4                          # gather instructions per batch (512)
TBL_ROWS = N_NODES + 4                # + unit tet with zero displacement (pads)

_cache = {}


def _configure(n_nodes, n_els, batches):
    """Test hook: shrink the problem (call before kernel())."""
    global N_NODES, N_ELS, ELS_PER_CORE, BATCHES, QUADS, EPC_PAD, TBL_ROWS
    N_NODES = n_nodes
    N_ELS = n_els
    ELS_PER_CORE = N_ELS // NC
    BATCHES = batches
    QUADS = BATCHES * QB
    EPC_PAD = QUADS * 128
    TBL_ROWS = N_NODES + 4
    assert EPC_PAD >= ELS_PER_CORE
    _cache.clear()


def _build_nc():
    import concourse.bass as bass
    import concourse.bacc as bacc
    import concourse.mybir as mybir
    import concourse.tile as tile

    f32 = mybir.dt.float32
    i32 = mybir.dt.int32
    P = 128

    nc = bacc.Bacc("TRN2", target_bir_lowering=False, debug=False)
    table = nc.dram_tensor("table", [TBL_ROWS, 8], f32, kind="ExternalInput").ap()
    idx = nc.dram_tensor("idx", [P, BATCHES * GPB], i32, kind="ExternalInput").ap()
    out = nc.dram_tensor("out", [P, 1], f32, kind="ExternalOutput").ap()

    with tile.TileContext(nc) as tc:
        with (
            tc.tile_pool(name="idxp", bufs=1) as idxp,
            tc.tile_pool(name="gp", bufs=3) as gp,
            tc.tile_pool(name="tp", bufs=1) as tp,
            tc.tile_pool(name="accp", bufs=1) as accp,
        ):
            acc = accp.tile([P, QB], f32)
            nc.vector.memset(acc[:], 0.0)

            # all offsets resident upfront: one big DMA instead of 31 small
            # ones, so every gather depends on a single already-satisfied sem
            idxall = idxp.tile([P, BATCHES * GPB], i32)
            nc.sync.dma_start(idxall[:], idx[:])

            def body(iv):
                G = gp.tile([P, GPB * 8], f32)
                for j in range(GPB):
                    nc.gpsimd.indirect_dma_start(
                        out=G[:, j * 8 : (j + 1) * 8],
                        out_offset=None,
                        in_=table[:],
                        in_offset=bass.IndirectOffsetOnAxis(
                            ap=idxall[:, iv * GPB + j : iv * GPB + j + 1], axis=0
                        ),
                    )

                # strided views: value of (slot k, field f) for the batch's
                # 128 quads: [P, QB] with free stride 32
                G3 = G[:].rearrange("p (q w) -> p q w", w=32)

                def V(k, f):
                    return G3[:, :, k * 8 + f]

                # temp pool: unique tag per temp so live ranges don't collide;
                # same tags across batches -> slots reused batch to batch
                tctr = [0]

                def T():
                    tctr[0] += 1
                    return tp.tile(
                        [P, QB], f32, name=f"tmp{tctr[0]}", tag=f"tmp{tctr[0]}"
                    )

                def sub(a, b):
                    o = T()
                    nc.vector.tensor_tensor(o[:], a, b, op=mybir.AluOpType.subtract)
                    return o

                def mul(a, b):
                    o = T()
                    nc.vector.tensor_tensor(o[:], a, b, op=mybir.AluOpType.mult)
                    return o

                def add(a, b):
                    o = T()
                    nc.vector.tensor_tensor(o[:], a, b, op=mybir.AluOpType.add)
                    return o

                # edge vectors e_k = x_k - x_0 ; du_k = u_k - u_0
                e = [[sub(V(k, f), V(0, f)) for f in range(3)] for k in (1, 2, 3)]
                du = [[sub(V(k, 3 + f), V(0, 3 + f)) for f in range(3)] for k in (1, 2, 3)]

                def cross(a, b):
                    o = []
                    for i in range(3):
                        j, k = (i + 1) % 3, (i + 2) % 3
                        m1 = mul(a[j][:], b[k][:])
                        m2 = mul(a[k][:], b[j][:])
                        o.append(sub(m1[:], m2[:]))
                    return o

                r1 = cross(e[1], e[2])   # e2 x e3
                r2 = cross(e[2], e[0])   # e3 x e1
                r3 = cross(e[0], e[1])   # e1 x e2
                r = [r1, r2, r3]

                # det = e1 . r1
                d0 = mul(e[0][0][:], r1[0][:])
                d1 = mul(e[0][1][:], r1[1][:])
                d2 = mul(e[0][2][:], r1[2][:])
                det = add(add(d0[:], d1[:])[:], d2[:])

                # P[i][j] = sum_k du_k[i] r_k[j]
                Pm = []
                for i in range(3):
                    row = []
                    for j in range(3):
                        m1 = mul(du[0][i][:], r[0][j][:])
                        m2 = mul(du[1][i][:], r[1][j][:])
                        m3 = mul(du[2][i][:], r[2][j][:])
                        row.append(add(add(m1[:], m2[:])[:], m3[:]))
                    Pm.append(row)

                # num = diag_sq + 0.5*off_sq + tr^2
                dsq0 = mul(Pm[0][0][:], Pm[0][0][:])
                dsq1 = mul(Pm[1][1][:], Pm[1][1][:])
                dsq2 = mul(Pm[2][2][:], Pm[2][2][:])
                diag = add(add(dsq0[:], dsq1[:])[:], dsq2[:])
                t01 = add(Pm[0][1][:], Pm[1][0][:])
                t02 = add(Pm[0][2][:], Pm[2][0][:])
                t12 = add(Pm[1][2][:], Pm[2][1][:])
                o0 = mul(t01[:], t01[:])
                o1 = mul(t02[:], t02[:])
                o2 = mul(t12[:], t12[:])
                off = add(add(o0[:], o1[:])[:], o2[:])
                # s1 = diag + 0.5*off
                s1 = T()
                nc.vector.scalar_tensor_tensor(
                    s1[:], off[:], 0.5, diag[:],
                    op0=mybir.AluOpType.mult, op1=mybir.AluOpType.add,
                )
                tr = add(add(Pm[0][0][:], Pm[1][1][:])[:], Pm[2][2][:])
                tr2 = mul(tr[:], tr[:])
                num = add(s1[:], tr2[:])

                # contribution = 0.5 * num / (6*det) = num * recip(12*det)
                det12 = T()
                nc.vector.tensor_scalar_mul(det12[:], det[:], 12.0)
                rec = T()
                nc.vector.reciprocal(rec[:], det12[:])
                contrib = mul(num[:], rec[:])
                nc.vector.tensor_tensor(
                    acc[:], acc[:], contrib[:], op=mybir.AluOpType.add
                )

            for b in range(BATCHES):
                body(b)

            red = accp.tile([P, 1], f32)
            nc.vector.tensor_reduce(
                red[:], acc[:], axis=mybir.AxisListType.X, op=mybir.AluOpType.add
            )
            nc.sync.dma_start(out[:], red[:])
    nc.compile()
    return nc


def _prepare_inputs(coords, us, conns):
    """Build the padded node table and per-core slot-major index arrays."""
    table = np.zeros((TBL_ROWS, 8), dtype=np.float32)
    table[:N_NODES, 0:3] = coords
    table[:N_NODES, 3:6] = us
    # pad element nodes: unit tet, zero displacement -> exactly zero energy
    table[N_NODES + 0, 0:3] = (0.0, 0.0, 0.0)
    table[N_NODES + 1, 0:3] = (1.0, 0.0, 0.0)
    table[N_NODES + 2, 0:3] = (0.0, 1.0, 0.0)
    table[N_NODES + 3, 0:3] = (0.0, 0.0, 1.0)

    pad_row = np.array(
        [N_NODES, N_NODES + 1, N_NODES + 2, N_NODES + 3], dtype=np.int32
    )
    idx_maps = []
    for c in range(NC):
        sh = conns[c * ELS_PER_CORE : (c + 1) * ELS_PER_CORE]
        full = np.empty((EPC_PAD, 4), dtype=np.int32)
        full[:ELS_PER_CORE] = sh
        full[ELS_PER_CORE:] = pad_row
        # element (b, q, p) = full[b*16384 + q*128 + p]
        # gather instruction j = q*4 + k gathers partition p <- slot k of elem q,p
        e4 = full.reshape(BATCHES, QB, 128, 4)        # [b, q, p, k]
        idx_arr = e4.transpose(2, 0, 1, 3).reshape(128, BATCHES * GPB)
        # ^ [p, b*GPB + q*4+k] : batch b, instruction j=q*4+k, partition p
        idx_maps.append(np.ascontiguousarray(idx_arr))
    return table, idx_maps


def kernel(coords, us, conns, t, state_old, dt):
    from concourse.bass_utils import run_bass_kernel_spmd

    coords = np.asarray(coords)
    us = np.asarray(us)
    conns = np.asarray(conns)

    if "nc" not in _cache:
        _cache["nc"] = _build_nc()
    nc = _cache["nc"]

    table, idx_maps = _prepare_inputs(coords, us, conns)
    in_maps = [{"table": table, "idx": idx_maps[c]} for c in range(NC)]
    res = run_bass_kernel_spmd(nc, in_maps, core_ids=list(range(NC)))
    partials = np.concatenate([r["out"].ravel() for r in res.results])
    pi = np.float32(partials.astype(np.float32).sum(dtype=np.float64))
    if np.isnan(partials).any():
        pi = np.float32(np.nan)
    return pi, np.asarray(state_old)
